# revision 4
# baseline (speedup 1.0000x reference)
"""Trainium2 Bass kernel for nn_AlgelogicNetwork (fuzzy rule matching -> softmax).

kernel(**inputs) takes the FULL unsharded inputs of reference.setup_inputs()
and returns the FULL output (softmax over M=16 rule strengths, (16,) float32).

The problem is tiny (<<1MB), so the whole computation is replicated on each of
the 8 NeuronCores (SPMD with identical inputs); core 0's output is returned.
The device program is a single-core raw-Bass kernel with manual semaphores:

  - Host packs all inputs into ONE [48, NPACK] f32 array (pure layout ops:
    transpose / reshape / tile / concat + constant identity/bias/ones columns).
    One DMA in, one DMA out.
  - Rule-premise pairs live at partition p = j*32 + m (j=0 -> rows 0:16,
    j=1 -> rows 32:48; rows 16:32 hold garbage that is never read) because
    compute-engine APs may only start at partitions 0/32/64/96.
  - match~[p,w] = sum_l sig[p,l]*(wm[w,l]^2 - 2 c[p,l] wm[w,l]) via one
    accumulated PE matmul pair (the sig*c^2 term is a per-p constant and
    argmin-invariant, so it is dropped).
  - argmin via reduce_min + is_equal one-hot; the captured-variable gather is
    one-hot * (mask*head @ wm) precomputed on the idle GPSIMD engine.
  - tail Linear folds its bias via an extended [tail|bias] layout; row norm
    via square+reduce; sqrt computed as exp(0.5*ln(x)) so that ALL activation
    functions (ln/exp) come from one ACT table (plus sigmoid), with both
    table loads prewarmed by dummy ops off the critical path.
  - softmax across partitions via identity-matmul transpose + exp accum.
"""
import numpy as np
import concourse.bass as bass
from concourse import library_config, mybir

F32 = mybir.dt.float32
M, J, I, L, W = 16, 2, 3, 2, 9
FREE = 464
NPACK = 161

# DMA'd columns (same as v1)
C_GNAT, C_HEAD, C_WREP, C_GT, C_CT, C_WMT = 0, 2, 8, 26, 74, 122
C_TAIL, C_TAILB, C_ID, C_BM5, C_ZERO = 131, 137, 139, 155, 156
# compute columns
C_SIGT, C_CST, C_WM2, C_HM = 168, 216, 264, 273
C_T1, C_T2, C_HWW = 280, 307, 334
C_MINQ, C_OH, C_PSEL2 = 361, 362, 372
C_PICK, C_PICKB, C_PROD3, C_CONCL = 399, 402, 405, 413
C_VSQ, C_P2 = 415, 417
C_E, C_S, C_SINV, C_OUT = 418, 434, 435, 436
C_JUNKA, C_JUNKB, C_MASK = 454, 455, 452
C_CAPX = 157          # [16,4]: cols 157:160 computed cap, col 160 const 1.0


def pack_inputs(state, constants, gammas, head_w, tail_w, tail_b):
    p = np.zeros((48, NPACK), np.float32)
    wm = np.asarray(state, np.float32).reshape(W, L)
    for j in range(J):
        r = slice(j * 32, j * 32 + 16)
        p[r, C_GNAT:C_GNAT + 2] = gammas[:, 1 + j, :]
        p[r, C_HEAD:C_HEAD + 6] = head_w[:, j].reshape(16, 6)          # free i*2+l
        p[0:2, C_GT + j * 32:C_GT + j * 32 + 16] = gammas[:, 1 + j, :].T
        p[0:2, C_CT + j * 32:C_CT + j * 32 + 16] = constants[:, j, :].T
    p[:, C_WREP:C_WREP + 18] = np.tile(wm.T.reshape(-1), (48, 1))      # l*9+w
    p[0:2, C_WMT:C_WMT + 9] = wm.T
    # tail_ext: [tail | tailb] per l -> free l*4+i, i=3 is the bias column
    te = np.concatenate([tail_w, tail_b[:, :, None]], axis=2)          # [16, 2, 4]
    p[0:16, C_TAIL:C_TAIL + 8] = te.reshape(16, 8)
    p[0:16, 160] = 1.0                                                 # cap_ext bias entry
    p[0:16, C_ID:C_ID + 16] = np.eye(16, dtype=np.float32)
    p[:, C_BM5] = -5.0
    p[:, C_ZERO] = 0.0
    return p


def build():
    nc = bass.Bass("TRN2", target_bir_lowering=False, debug=False)
    packed = nc.dram_tensor("packed", [48, NPACK], F32, kind="ExternalInput")
    y = nc.dram_tensor("y", [1, 16], F32, kind="ExternalOutput")

    al = mybir.AluOpType
    af = mybir.ActivationFunctionType

    with (
        nc.sbuf_tensor("sb", [128, FREE], F32) as sb,
        nc.psum_tensor("mq", [48, 9], F32) as mq,
        nc.psum_tensor("pnt", [1, 16], F32) as pnt,
        nc.semaphore("s_dma") as s_dma,
        nc.semaphore("s_act") as s_act,
        nc.semaphore("s_dve") as s_dve,
        nc.semaphore("s_pe") as s_pe,
        nc.semaphore("s_out") as s_out,
        nc.semaphore("s_pool") as s_pool,
    ):
        def A(r0, nr, c0, dims):
            return bass.AP(sb, r0 * FREE + c0, [[FREE, nr]] + [list(d) for d in dims])

        MQ = lambda: bass.AP(mq, 0, [[9, 48], [1, 9]])
        PNT = lambda: bass.AP(pnt, 0, [[16, 1], [1, 16]])

        sems = {"ACT": s_act, "DVE": s_dve, "PE": s_pe, "DMA": s_dma,
                "OUT": s_out, "POOL": s_pool}
        counts = {"ACT": 0, "DVE": 0, "PE": 0, "POOL": 0}
        waited = {k: {} for k in ("ACT", "DVE", "PE", "SP", "POOL")}

        def emit(ekey, engine, build_fn, deps=(), inc=True):
            need = {}
            if ekey in counts and counts[ekey] > 0:
                need[ekey] = counts[ekey]
            for sk, v in deps:
                if sk == ekey:
                    continue
                need[sk] = max(need.get(sk, 0), v)
            fresh = [(sk, v) for sk, v in need.items() if waited[ekey].get(sk, 0) < v]
            for sk, v in fresh[1:]:
                engine.wait_ge(sems[sk], v)
            inst = build_fn()
            for sk, v in fresh[:1]:
                inst._wait_ge(sems[sk], v)
            for sk, v in fresh:
                waited[ekey][sk] = v
            if inc and ekey in counts:
                counts[ekey] += 1
                inst.then_inc(sems[ekey], 1)
            return inst

        with nc.Block() as block:

            @block.sync
            def _(sync):
                sync.dma_start(
                    out=A(0, 48, 0, [(1, NPACK)]),
                    in_=bass.AP(packed, 0, [[NPACK, 48], [1, NPACK]]),
                ).then_inc(s_dma, 16)

            @block.vector
            def _(vector):
                # v1: junk=1.0 for the dummy activation inputs
                emit("DVE", vector, lambda: vector.memset(A(0, 1, C_JUNKA, [(1, 2)]), 1.0))
                # v2: wm2T = wmT^2
                emit("DVE", vector, lambda: vector.tensor_mul(
                    A(0, 2, C_WM2, [(1, 9)]), A(0, 2, C_WMT, [(1, 9)]),
                    A(0, 2, C_WMT, [(1, 9)]),
                ), deps=[("DMA", 16)])
                # v3: csT = (sigT * -2) * cT  -- right after sigmoid: feeds m1b
                emit("DVE", vector, lambda: vector.scalar_tensor_tensor(
                    A(0, 2, C_CST, [(1, 48)]), A(0, 2, C_SIGT, [(1, 48)]), -2.0,
                    A(0, 2, C_CT, [(1, 48)]), op0=al.mult, op1=al.mult,
                ), deps=[("ACT", 2)])

            @block.gpsimd
            def _(g):
                # p1: mask = (g_nat > 0.5)  [48, 2] (stored at C_MASK)
                emit("POOL", g, lambda: g.tensor_scalar(
                    A(0, 48, C_MASK, [(1, 2)]), A(0, 48, C_GNAT, [(1, 2)]),
                    0.5, None, al.is_gt,
                ), deps=[("DMA", 16)])
                # p2: hm = mask (bcast i) * head_nat, stored [l, i] (l*3+i)
                emit("POOL", g, lambda: g.tensor_mul(
                    A(0, 48, C_HM, [(3, 2), (1, 3)]),
                    A(0, 48, C_MASK, [(1, 2), (0, 3)]),
                    A(0, 48, C_HEAD, [(2, 3), (1, 2)]).transpose([0, 2, 1]),
                ))
                # p3: t1 = hm[l=0, i] (bcast w) * wmrep[l=0, w] (bcast i)
                emit("POOL", g, lambda: g.tensor_mul(
                    A(0, 48, C_T1, [(9, 3), (1, 9)]),
                    A(0, 48, C_HM, [(1, 3), (0, 9)]),
                    A(0, 48, C_WREP, [(0, 3), (1, 9)]),
                ))
                # p4: t2 = same for l=1
                emit("POOL", g, lambda: g.tensor_mul(
                    A(0, 48, C_T2, [(9, 3), (1, 9)]),
                    A(0, 48, C_HM + 3, [(1, 3), (0, 9)]),
                    A(0, 48, C_WREP + 9, [(0, 3), (1, 9)]),
                ))
                # p5: hww = t1 + t2   [48, 27]
                emit("POOL", g, lambda: g.tensor_add(
                    A(0, 48, C_HWW, [(1, 27)]), A(0, 48, C_T1, [(1, 27)]),
                    A(0, 48, C_T2, [(1, 27)]),
                ))

            @block.scalar
            def _(scalar):
                # a1: dummy sigmoid -> loads sigmoid table during the DMA
                emit("ACT", scalar, lambda: scalar.activation(
                    A(0, 1, C_JUNKA, [(1, 1)]), A(0, 1, C_JUNKA, [(1, 1)]),
                    af.Sigmoid, bias=A(0, 1, C_JUNKA, [(1, 1)]), scale=1.0,
                ), deps=[("DVE", 1)])
                # a2: sigT = sigmoid(10*gT - 5)
                emit("ACT", scalar, lambda: scalar.activation(
                    A(0, 2, C_SIGT, [(1, 48)]), A(0, 2, C_GT, [(1, 48)]),
                    af.Sigmoid, bias=A(0, 2, C_BM5, [(1, 1)]), scale=10.0,
                ), deps=[("DMA", 16)])
                # a3: dummy ln -> loads ln/exp table during the match chain
                emit("ACT", scalar, lambda: scalar.activation(
                    A(0, 1, C_JUNKB, [(1, 1)]), A(0, 1, C_JUNKB, [(1, 1)]),
                    af.Ln, bias=A(0, 1, C_ZERO, [(1, 1)]), scale=1.0,
                ), deps=[("DVE", 1)])

            @block.tensor
            def _(tensor):
                # m1a+m1b: match = sigT.T@wm2T + csT.T@wmT -> PSUM [48, 9]
                emit("PE", tensor, lambda: tensor.matmul(
                    MQ(), A(0, 2, C_SIGT, [(1, 48)]), A(0, 2, C_WM2, [(1, 9)]),
                    start=True, stop=False,
                ), deps=[("ACT", 2), ("DVE", 2)])
                emit("PE", tensor, lambda: tensor.matmul(
                    MQ(), A(0, 2, C_CST, [(1, 48)]), A(0, 2, C_WMT, [(1, 9)]),
                    start=False, stop=True,
                ), deps=[("DVE", 3)])

            @block.vector
            def _(vector):
                # v8: min over w
                emit("DVE", vector, lambda: vector.tensor_reduce(
                    A(0, 48, C_MINQ, [(1, 1)]), MQ(),
                    axis=mybir.AxisListType.X, op=al.min,
                ), deps=[("PE", 2)])
                # v9: onehot = (match == min)
                emit("DVE", vector, lambda: vector.tensor_scalar(
                    A(0, 48, C_OH, [(1, 9)]), MQ(),
                    A(0, 48, C_MINQ, [(1, 1)]), None, al.is_equal,
                ))
                # v10: psel2 = onehot (bcast i) * hww   [48, 3, 9]
                emit("DVE", vector, lambda: vector.tensor_mul(
                    A(0, 48, C_PSEL2, [(9, 3), (1, 9)]),
                    A(0, 48, C_OH, [(0, 3), (1, 9)]),
                    A(0, 48, C_HWW, [(9, 3), (1, 9)]),
                ), deps=[("POOL", 5)])
                # v11: picked = sum_w psel2 -> [48, 3]
                emit("DVE", vector, lambda: vector.tensor_reduce(
                    A(0, 48, C_PICK, [(1, 3)]), A(0, 48, C_PSEL2, [(9, 3), (1, 9)]),
                    axis=mybir.AxisListType.X, op=al.add,
                ))
                # v12/v13: cap = picked[j=0] + picked[j=1]
                emit("DVE", vector, lambda: vector.tensor_copy(
                    A(0, 16, C_PICKB, [(1, 3)]), A(32, 16, C_PICK, [(1, 3)]),
                ))
                emit("DVE", vector, lambda: vector.tensor_add(
                    A(0, 16, C_CAPX, [(1, 3)]), A(0, 16, C_PICK, [(1, 3)]),
                    A(0, 16, C_PICKB, [(1, 3)]),
                ))
                # v14: prod3 = tail_ext * cap_ext (bcast over l; i=3 is bias*1)
                emit("DVE", vector, lambda: vector.tensor_mul(
                    A(0, 16, C_PROD3, [(4, 2), (1, 4)]),
                    A(0, 16, C_TAIL, [(4, 2), (1, 4)]),
                    A(0, 16, C_CAPX, [(0, 2), (1, 4)]),
                ))
                # v15: concl = sum_i prod3 (bias included)
                emit("DVE", vector, lambda: vector.tensor_reduce(
                    A(0, 16, C_CONCL, [(1, 2)]), A(0, 16, C_PROD3, [(4, 2), (1, 4)]),
                    axis=mybir.AxisListType.X, op=al.add,
                ))
                # v17: vsq = concl^2 with P2 = sum_l accumulated in one op
                emit("DVE", vector, lambda: vector.scalar_tensor_tensor(
                    A(0, 16, C_VSQ, [(1, 2)]), A(0, 16, C_CONCL, [(1, 2)]), 1.0,
                    A(0, 16, C_CONCL, [(1, 2)]), op0=al.mult, op1=al.mult,
                    accum_out=A(0, 16, C_P2, [(1, 1)]),
                ))

            @block.tensor
            def _(tensor):
                # m2: P2T = P2.T @ I16 -> PSUM [1, 16]
                emit("PE", tensor, lambda: tensor.matmul(
                    PNT(), A(0, 16, C_P2, [(1, 1)]), A(0, 16, C_ID, [(1, 16)]),
                    start=True, stop=True,
                ), deps=[("DVE", 12)])

            @block.scalar
            def _(scalar):
                # a4: lnx = ln(P2T)   [1,16]  (reuse C_VSQ row0 as scratch? no: C_JUNK area)
                emit("ACT", scalar, lambda: scalar.activation(
                    A(0, 1, C_E, [(1, 16)]), PNT(),
                    af.Ln, bias=A(0, 1, C_ZERO, [(1, 1)]), scale=1.0,
                ), deps=[("PE", 3)])
                # a5: P = exp(0.5*lnx) = sqrt(P2)   [1,16] -> overwrite in place? new col
                emit("ACT", scalar, lambda: scalar.activation(
                    A(0, 1, C_OUT, [(1, 16)]), A(0, 1, C_E, [(1, 16)]),
                    af.Exp, bias=A(0, 1, C_ZERO, [(1, 1)]), scale=0.5,
                ))
                # a6: e = exp(P), S = sum e
                emit("ACT", scalar, lambda: scalar.activation(
                    A(0, 1, C_E, [(1, 16)]), A(0, 1, C_OUT, [(1, 16)]),
                    af.Exp, bias=A(0, 1, C_ZERO, [(1, 1)]), scale=1.0,
                    accum_out=A(0, 1, C_S, [(1, 1)]),
                ))

            @block.vector
            def _(vector):
                # v19/v20: out = e / S
                emit("DVE", vector, lambda: vector.reciprocal(
                    A(0, 1, C_SINV, [(1, 1)]), A(0, 1, C_S, [(1, 1)]),
                ), deps=[("ACT", 6)])
                emit("DVE", vector, lambda: vector.tensor_scalar(
                    A(0, 1, C_OUT, [(1, 16)]), A(0, 1, C_E, [(1, 16)]),
                    A(0, 1, C_SINV, [(1, 1)]), None, al.mult,
                ))

            @block.sync
            def _(sync):
                emit("SP", sync, lambda: sync.dma_start(
                    out=bass.AP(y, 0, [[16, 1], [1, 16]]),
                    in_=A(0, 1, C_OUT, [(1, 16)]),
                ), deps=[("DVE", 14)], inc=False).then_inc(s_out, 16)
                sync.wait_ge(s_out, 16)

    return nc




_NC = None


def _get_nc():
    global _NC
    if _NC is None:
        _NC = build()
    return _NC


def kernel(state, constants, gammas, head_w, tail_w, tail_b):
    from concourse.bass_utils import run_bass_kernel_spmd

    state = np.asarray(state, np.float32)
    constants = np.asarray(constants, np.float32)
    gammas = np.asarray(gammas, np.float32)
    head_w = np.asarray(head_w, np.float32)
    tail_w = np.asarray(tail_w, np.float32)
    tail_b = np.asarray(tail_b, np.float32)

    packed = pack_inputs(state, constants, gammas, head_w, tail_w, tail_b)
    nc = _get_nc()
    in_maps = [{"packed": packed} for _ in range(8)]
    res = run_bass_kernel_spmd(nc, in_maps, core_ids=list(range(8)))
    return res.results[0]["y"].reshape(M).astype(np.float32)


# revision 9
# speedup vs baseline: 1.0621x; 1.0621x over previous
"""Trainium2 Bass kernel for nn_AlgelogicNetwork (fuzzy rule matching -> softmax).

kernel(**inputs) takes the FULL unsharded inputs of reference.setup_inputs()
and returns the FULL output (softmax over M=16 rule strengths, (16,) float32).

The problem is tiny (<<1MB), so the whole computation is replicated on each of
the 8 NeuronCores (SPMD with identical inputs); core 0's output is returned.
The device program is a single-core raw-Bass kernel with manual semaphores:

  - Host packs all inputs into ONE [48, NPACK] f32 array (pure layout ops:
    transpose / reshape / tile / concat + constant identity/bias/ones columns).
    One DMA in, one DMA out.
  - Rule-premise pairs live at partition p = j*32 + m (j=0 -> rows 0:16,
    j=1 -> rows 32:48; rows 16:32 hold garbage that is never read) because
    compute-engine APs may only start at partitions 0/32/64/96.
  - match~[p,w] = sum_l sig[p,l]*(wm[w,l]^2 - 2 c[p,l] wm[w,l]) via one
    accumulated PE matmul pair (the sig*c^2 term is a per-p constant and
    argmin-invariant, so it is dropped).
  - argmin via reduce_min + is_equal one-hot; the captured-variable gather is
    one-hot * (mask*head @ wm) precomputed on the idle GPSIMD engine.
  - the argmin one-hot is fused into the gather: one scalar_tensor_tensor
    computes (match == min) * hww with match read from PSUM broadcast over i.
  - tail Linear folds its bias via an extended [tail|bias] layout; the row
    norm's square+sum is one scalar_tensor_tensor with accum_out; sqrt is
    computed as exp(0.5*ln(x)) so that ALL activation functions (ln/exp)
    come from one ACT table (plus sigmoid), with both table loads prewarmed
    by dummy ops off the critical path.
  - softmax across partitions via identity-matmul transpose + exp accum.
  - no explicit wait on the output-DMA semaphore: the Block-exit drain
    blocks until the HWDGE queue is empty (validated over repeated HW runs),
    saving the DMA-sem propagation delay.
  - cost-model (TimelineSim) makespan: ~10.2 us (from ~11.3 us baseline);
    bounded by the input-DMA fixed path (~3.3 us), the serial compute chain
    (~4.6 us), and the output-DMA fixed path (~2.3 us).
"""
import numpy as np
import concourse.bass as bass
from concourse import library_config, mybir

F32 = mybir.dt.float32
M, J, I, L, W = 16, 2, 3, 2, 9
FREE = 512
NPACK = 161

# DMA'd columns (same as v1)
C_GNAT, C_HEAD, C_WREP, C_GT, C_CT, C_WMT = 0, 2, 8, 26, 74, 122
C_TAIL, C_TAILB, C_ID, C_BM5, C_ZERO = 131, 137, 139, 155, 156
# compute columns
C_SIGT, C_CST, C_WM2, C_HM = 168, 216, 264, 273
C_T1, C_T2, C_HWW = 280, 307, 334
C_MINQ, C_OH, C_PSEL2 = 361, 362, 372
C_PICK, C_PICKB, C_PROD3, C_CONCL = 399, 402, 405, 413
C_VSQ, C_P2 = 415, 417
C_E, C_S, C_SINV, C_OUT = 418, 434, 435, 436
C_JUNKA, C_JUNKB, C_MASK = 454, 455, 452
C_M2C = 457           # [2,48] scratch: -2*cT
C_CAPX = 157          # [16,4]: cols 157:160 computed cap, col 160 const 1.0


def pack_inputs(state, constants, gammas, head_w, tail_w, tail_b):
    p = np.zeros((48, NPACK), np.float32)
    wm = np.asarray(state, np.float32).reshape(W, L)
    for j in range(J):
        r = slice(j * 32, j * 32 + 16)
        p[r, C_GNAT:C_GNAT + 2] = gammas[:, 1 + j, :]
        p[r, C_HEAD:C_HEAD + 6] = head_w[:, j].reshape(16, 6)          # free i*2+l
        p[0:2, C_GT + j * 32:C_GT + j * 32 + 16] = gammas[:, 1 + j, :].T
        p[0:2, C_CT + j * 32:C_CT + j * 32 + 16] = constants[:, j, :].T
    p[:, C_WREP:C_WREP + 18] = np.tile(wm.T.reshape(-1), (48, 1))      # l*9+w
    p[0:2, C_WMT:C_WMT + 9] = wm.T
    # tail_ext: [tail | tailb] per l -> free l*4+i, i=3 is the bias column
    te = np.concatenate([tail_w, tail_b[:, :, None]], axis=2)          # [16, 2, 4]
    p[0:16, C_TAIL:C_TAIL + 8] = te.reshape(16, 8)
    p[0:16, 160] = 1.0                                                 # cap_ext bias entry
    p[0:16, C_ID:C_ID + 16] = np.eye(16, dtype=np.float32)
    p[:, C_BM5] = -5.0
    p[:, C_ZERO] = 0.0
    return p


def build():
    nc = bass.Bass("TRN2", target_bir_lowering=False, debug=False)
    packed = nc.dram_tensor("packed", [48, NPACK], F32, kind="ExternalInput")
    y = nc.dram_tensor("y", [1, 16], F32, kind="ExternalOutput")

    al = mybir.AluOpType
    af = mybir.ActivationFunctionType

    with (
        nc.sbuf_tensor("sb", [128, FREE], F32) as sb,
        nc.psum_tensor("mq", [48, 9], F32) as mq,
        nc.psum_tensor("pnt", [1, 16], F32) as pnt,
        nc.semaphore("s_dma") as s_dma,
        nc.semaphore("s_act") as s_act,
        nc.semaphore("s_dve") as s_dve,
        nc.semaphore("s_pe") as s_pe,
        nc.semaphore("s_out") as s_out,
        nc.semaphore("s_pool") as s_pool,
    ):
        def A(r0, nr, c0, dims):
            return bass.AP(sb, r0 * FREE + c0, [[FREE, nr]] + [list(d) for d in dims])

        MQ = lambda: bass.AP(mq, 0, [[9, 48], [1, 9]])
        PNT = lambda: bass.AP(pnt, 0, [[16, 1], [1, 16]])

        sems = {"ACT": s_act, "DVE": s_dve, "PE": s_pe, "DMA": s_dma,
                "OUT": s_out, "POOL": s_pool}
        counts = {"ACT": 0, "DVE": 0, "PE": 0, "POOL": 0}
        waited = {k: {} for k in ("ACT", "DVE", "PE", "SP", "POOL")}

        def emit(ekey, engine, build_fn, deps=(), inc=True):
            need = {}
            if ekey in counts and counts[ekey] > 0:
                need[ekey] = counts[ekey]
            for sk, v in deps:
                if sk == ekey:
                    continue
                need[sk] = max(need.get(sk, 0), v)
            fresh = [(sk, v) for sk, v in need.items() if waited[ekey].get(sk, 0) < v]
            for sk, v in fresh[1:]:
                engine.wait_ge(sems[sk], v)
            inst = build_fn()
            for sk, v in fresh[:1]:
                inst._wait_ge(sems[sk], v)
            for sk, v in fresh:
                waited[ekey][sk] = v
            if inc and ekey in counts:
                counts[ekey] += 1
                inst.then_inc(sems[ekey], 1)
            return inst

        with nc.Block() as block:

            @block.sync
            def _(sync):
                sync.dma_start(
                    out=A(0, 48, 0, [(1, NPACK)]),
                    in_=bass.AP(packed, 0, [[NPACK, 48], [1, NPACK]]),
                ).then_inc(s_dma, 16)

            @block.vector
            def _(vector):
                # v1: junk=1.0 for the dummy activation inputs
                emit("DVE", vector, lambda: vector.memset(A(0, 1, C_JUNKA, [(1, 2)]), 1.0))
                # v2: wm2T = wmT^2
                emit("DVE", vector, lambda: vector.tensor_mul(
                    A(0, 2, C_WM2, [(1, 9)]), A(0, 2, C_WMT, [(1, 9)]),
                    A(0, 2, C_WMT, [(1, 9)]),
                ), deps=[("DMA", 16)])
                # v3: m2c = -2*cT (DMA-only dep; runs during the sigmoid)
                emit("DVE", vector, lambda: vector.tensor_scalar(
                    A(0, 2, C_M2C, [(1, 48)]), A(0, 2, C_CT, [(1, 48)]),
                    -2.0, None, al.mult,
                ))
                # v4: csT = sigT * m2c -- right after sigmoid: feeds m1b
                emit("DVE", vector, lambda: vector.tensor_mul(
                    A(0, 2, C_CST, [(1, 48)]), A(0, 2, C_SIGT, [(1, 48)]),
                    A(0, 2, C_M2C, [(1, 48)]),
                ), deps=[("ACT", 2)])

            @block.gpsimd
            def _(g):
                # p1: mask = (g_nat > 0.5)  [48, 2] (stored at C_MASK)
                emit("POOL", g, lambda: g.tensor_scalar(
                    A(0, 48, C_MASK, [(1, 2)]), A(0, 48, C_GNAT, [(1, 2)]),
                    0.5, None, al.is_gt,
                ), deps=[("DMA", 16)])
                # p2: hm = mask (bcast i) * head_nat, stored [l, i] (l*3+i)
                emit("POOL", g, lambda: g.tensor_mul(
                    A(0, 48, C_HM, [(3, 2), (1, 3)]),
                    A(0, 48, C_MASK, [(1, 2), (0, 3)]),
                    A(0, 48, C_HEAD, [(2, 3), (1, 2)]).transpose([0, 2, 1]),
                ))
                # p3: t1 = hm[l=0, i] (bcast w) * wmrep[l=0, w] (bcast i)
                emit("POOL", g, lambda: g.tensor_mul(
                    A(0, 48, C_T1, [(9, 3), (1, 9)]),
                    A(0, 48, C_HM, [(1, 3), (0, 9)]),
                    A(0, 48, C_WREP, [(0, 3), (1, 9)]),
                ))
                # p4: t2 = same for l=1
                emit("POOL", g, lambda: g.tensor_mul(
                    A(0, 48, C_T2, [(9, 3), (1, 9)]),
                    A(0, 48, C_HM + 3, [(1, 3), (0, 9)]),
                    A(0, 48, C_WREP + 9, [(0, 3), (1, 9)]),
                ))
                # p5: hww = t1 + t2   [48, 27]
                emit("POOL", g, lambda: g.tensor_add(
                    A(0, 48, C_HWW, [(1, 27)]), A(0, 48, C_T1, [(1, 27)]),
                    A(0, 48, C_T2, [(1, 27)]),
                ))

            @block.scalar
            def _(scalar):
                # a1: dummy sigmoid -> loads sigmoid table during the DMA
                emit("ACT", scalar, lambda: scalar.activation(
                    A(0, 1, C_JUNKA, [(1, 1)]), A(0, 1, C_JUNKA, [(1, 1)]),
                    af.Sigmoid, bias=A(0, 1, C_JUNKA, [(1, 1)]), scale=1.0,
                ), deps=[("DVE", 1)])
                # a2: sigT = sigmoid(10*gT - 5)
                emit("ACT", scalar, lambda: scalar.activation(
                    A(0, 2, C_SIGT, [(1, 48)]), A(0, 2, C_GT, [(1, 48)]),
                    af.Sigmoid, bias=A(0, 2, C_BM5, [(1, 1)]), scale=10.0,
                ), deps=[("DMA", 16)])
                # a3: dummy ln -> loads ln/exp table during the match chain
                emit("ACT", scalar, lambda: scalar.activation(
                    A(0, 1, C_JUNKB, [(1, 1)]), A(0, 1, C_JUNKB, [(1, 1)]),
                    af.Ln, bias=A(0, 1, C_ZERO, [(1, 1)]), scale=1.0,
                ), deps=[("DVE", 1)])

            @block.tensor
            def _(tensor):
                # m1a+m1b: match = sigT.T@wm2T + csT.T@wmT -> PSUM [48, 9]
                emit("PE", tensor, lambda: tensor.matmul(
                    MQ(), A(0, 2, C_SIGT, [(1, 48)]), A(0, 2, C_WM2, [(1, 9)]),
                    start=True, stop=False,
                ), deps=[("ACT", 2), ("DVE", 2)])
                emit("PE", tensor, lambda: tensor.matmul(
                    MQ(), A(0, 2, C_CST, [(1, 48)]), A(0, 2, C_WMT, [(1, 9)]),
                    start=False, stop=True,
                ), deps=[("DVE", 4)])

            @block.vector
            def _(vector):
                # v8: min over w
                emit("DVE", vector, lambda: vector.tensor_reduce(
                    A(0, 48, C_MINQ, [(1, 1)]), MQ(),
                    axis=mybir.AxisListType.X, op=al.min,
                ), deps=[("PE", 2)])
                # v9: psel2 = (match == min, bcast i) * hww   [48, 3, 9]
                emit("DVE", vector, lambda: vector.scalar_tensor_tensor(
                    A(0, 48, C_PSEL2, [(9, 3), (1, 9)]),
                    bass.AP(mq, 0, [[9, 48], [0, 3], [1, 9]]),
                    A(0, 48, C_MINQ, [(1, 1)]),
                    A(0, 48, C_HWW, [(9, 3), (1, 9)]),
                    op0=al.is_equal, op1=al.mult,
                ), deps=[("POOL", 5)])
                # v11: picked = sum_w psel2 -> [48, 3]
                emit("DVE", vector, lambda: vector.tensor_reduce(
                    A(0, 48, C_PICK, [(1, 3)]), A(0, 48, C_PSEL2, [(9, 3), (1, 9)]),
                    axis=mybir.AxisListType.X, op=al.add,
                ))
                # v12/v13: cap = picked[j=0] + picked[j=1]
                emit("DVE", vector, lambda: vector.tensor_copy(
                    A(0, 16, C_PICKB, [(1, 3)]), A(32, 16, C_PICK, [(1, 3)]),
                ))
                emit("DVE", vector, lambda: vector.tensor_add(
                    A(0, 16, C_CAPX, [(1, 3)]), A(0, 16, C_PICK, [(1, 3)]),
                    A(0, 16, C_PICKB, [(1, 3)]),
                ))
                # v14: prod3 = tail_ext * cap_ext (bcast over l; i=3 is bias*1)
                emit("DVE", vector, lambda: vector.tensor_mul(
                    A(0, 16, C_PROD3, [(4, 2), (1, 4)]),
                    A(0, 16, C_TAIL, [(4, 2), (1, 4)]),
                    A(0, 16, C_CAPX, [(0, 2), (1, 4)]),
                ))
                # v15: concl = sum_i prod3 (bias included)
                emit("DVE", vector, lambda: vector.tensor_reduce(
                    A(0, 16, C_CONCL, [(1, 2)]), A(0, 16, C_PROD3, [(4, 2), (1, 4)]),
                    axis=mybir.AxisListType.X, op=al.add,
                ))
                # v17: vsq = concl^2 with P2 = sum_l accumulated in one op
                emit("DVE", vector, lambda: vector.scalar_tensor_tensor(
                    A(0, 16, C_VSQ, [(1, 2)]), A(0, 16, C_CONCL, [(1, 2)]), 1.0,
                    A(0, 16, C_CONCL, [(1, 2)]), op0=al.mult, op1=al.mult,
                    accum_out=A(0, 16, C_P2, [(1, 1)]),
                ))

            @block.tensor
            def _(tensor):
                # m2: P2T = P2.T @ I16 -> PSUM [1, 16]
                emit("PE", tensor, lambda: tensor.matmul(
                    PNT(), A(0, 16, C_P2, [(1, 1)]), A(0, 16, C_ID, [(1, 16)]),
                    start=True, stop=True,
                ), deps=[("DVE", 12)])

            @block.scalar
            def _(scalar):
                # a4: lnx = ln(P2T)   [1,16]  (reuse C_VSQ row0 as scratch? no: C_JUNK area)
                emit("ACT", scalar, lambda: scalar.activation(
                    A(0, 1, C_E, [(1, 16)]), PNT(),
                    af.Ln, bias=A(0, 1, C_ZERO, [(1, 1)]), scale=1.0,
                ), deps=[("PE", 3)])
                # a5: P = exp(0.5*lnx) = sqrt(P2)   [1,16] -> overwrite in place? new col
                emit("ACT", scalar, lambda: scalar.activation(
                    A(0, 1, C_OUT, [(1, 16)]), A(0, 1, C_E, [(1, 16)]),
                    af.Exp, bias=A(0, 1, C_ZERO, [(1, 1)]), scale=0.5,
                ))
                # a6: e = exp(P), S = sum e
                emit("ACT", scalar, lambda: scalar.activation(
                    A(0, 1, C_E, [(1, 16)]), A(0, 1, C_OUT, [(1, 16)]),
                    af.Exp, bias=A(0, 1, C_ZERO, [(1, 1)]), scale=1.0,
                    accum_out=A(0, 1, C_S, [(1, 1)]),
                ))

            @block.vector
            def _(vector):
                # v19/v20: out = e / S
                emit("DVE", vector, lambda: vector.reciprocal(
                    A(0, 1, C_SINV, [(1, 1)]), A(0, 1, C_S, [(1, 1)]),
                ), deps=[("ACT", 6)])
                emit("DVE", vector, lambda: vector.tensor_scalar(
                    A(0, 1, C_OUT, [(1, 16)]), A(0, 1, C_E, [(1, 16)]),
                    A(0, 1, C_SINV, [(1, 1)]), None, al.mult,
                ))

            @block.sync
            def _(sync):
                emit("SP", sync, lambda: sync.dma_start(
                    out=bass.AP(y, 0, [[16, 1], [1, 16]]),
                    in_=A(0, 1, C_OUT, [(1, 16)]),
                ), deps=[("DVE", 14)], inc=False).then_inc(s_out, 16)

    return nc




_NC = None


def _get_nc():
    global _NC
    if _NC is None:
        _NC = build()
    return _NC


def _default_inputs():
    """Regenerate setup_inputs()'s non-state parameters (jax key(0) recipe) in
    case the harness only supplies `state` (spec.json lists only state in
    input_specs)."""
    import jax
    import jax.numpy as jnp
    key = jax.random.key(0)
    ks = jax.random.split(key, 6)
    bL = 1.0 / np.sqrt(L)
    bI = 1.0 / np.sqrt(I)
    return dict(
        state=jax.random.normal(ks[0], (1, W * L), dtype=jnp.float32),
        constants=jax.random.uniform(ks[1], (M, J + 1, L), minval=-1.0, maxval=1.0, dtype=jnp.float32),
        gammas=jax.random.uniform(ks[2], (M, J + 1, L), minval=0.0, maxval=1.0, dtype=jnp.float32),
        head_w=jax.random.uniform(ks[3], (M, J, I, L), minval=-bL, maxval=bL, dtype=jnp.float32),
        tail_w=jax.random.uniform(ks[4], (M, L, I), minval=-bI, maxval=bI, dtype=jnp.float32),
        tail_b=jax.random.uniform(ks[5], (M, L), minval=-bI, maxval=bI, dtype=jnp.float32),
    )


def kernel(state=None, constants=None, gammas=None, head_w=None, tail_w=None,
           tail_b=None, **_unused):
    from concourse.bass_utils import run_bass_kernel_spmd

    if any(v is None for v in (state, constants, gammas, head_w, tail_w, tail_b)):
        d = _default_inputs()
        state = d["state"] if state is None else state
        constants = d["constants"] if constants is None else constants
        gammas = d["gammas"] if gammas is None else gammas
        head_w = d["head_w"] if head_w is None else head_w
        tail_w = d["tail_w"] if tail_w is None else tail_w
        tail_b = d["tail_b"] if tail_b is None else tail_b

    state = np.asarray(state, np.float32)
    constants = np.asarray(constants, np.float32)
    gammas = np.asarray(gammas, np.float32)
    head_w = np.asarray(head_w, np.float32)
    tail_w = np.asarray(tail_w, np.float32)
    tail_b = np.asarray(tail_b, np.float32)

    packed = pack_inputs(state, constants, gammas, head_w, tail_w, tail_b)
    nc = _get_nc()
    in_maps = [{"packed": packed} for _ in range(8)]
    res = run_bass_kernel_spmd(nc, in_maps, core_ids=list(range(8)))
    return res.results[0]["y"].reshape(M).astype(np.float32)
